# revision 31
# baseline (speedup 1.0000x reference)
"""Trainium2 Bass kernel for dilated sliding-window attention (AttnWrapper).

Reference computation (all fp32):
  combined = [begin | main | end]                       # [8256, 768]
  keys[t]  = combined[t + 32 + off], off in +-{4..32}   # 16 dilated window keys
  q = (main @ wq.T + bq) * 96**-0.5
  k/v = keys @ w{k,v}.T + b{k,v}
  attn = softmax(q.k), ctx = attn.v, out = [main | ctx @ wo.T + bo]

Sharding: tokens across 8 cores (1024 each) with a 64-row halo of the
combined buffer; weights replicated. Each core computes attn_outT
[768, 1024] in bf16; the host transposes and concatenates with main.

Device-side math notes:
 - All four projections run as fp8e4m3 matmuls in DoubleRow perf mode
   (K=256 per instruction): x and the weights are cast to fp8 on the
   host. Weights are pre-scaled by 2^6 so their ~0.02-std values land in
   fp8's normal range; the scale is unwound in the PSUM drain copies
   (v) or absorbed downstream (q.k scores carry 2^12, exp uses
   scale=2^-12; out drain scales by 2^-6).
 - bk dropped (softmax shift invariance); bv folded into bo' = wo@bv + bo;
   bq applied in the q PSUM-drain copy (it is zero for this problem).
 - q/k tensors and attention probabilities/V are bf16; normalized ctx is
   cast to fp8 for the DoubleRow out-projection (pairs of heads, K=192).
 - Scores are computed transposed (S[key, token]) in groups of 512 tokens
   x 576 keys split into key-chunks of (128x4,64); each chunk's matmul
   covers only its valid token window.
 - A ones-column appended to each V head block makes the ctx matmul also
   produce the softmax denominator (row 96 of the ctx PSUM tile); 1/denom
   via reciprocal_approx_fast on DVE (Ln/Exp on ACT would thrash the
   activation table against the score exps), broadcast on GpSimd, and the
   ctx drain fuses the multiply + fp8 cast in one DVE scalar_tensor_tensor.
 - Input DMAs are split across the sync/scalar (HWDGE) and gpsimd (SWDGE)
   queues (each queue sustains only ~22-50GB/s) so x and the weights land
   early; attention for both token groups is emitted interleaved so the
   PE always has independent work.
"""

import numpy as np

EMBED_DIM = 768
NUM_HEADS = 8
HEAD_DIM = 96
OVERLAP = 32
HALO = 2 * OVERLAP          # 64 extra combined rows per core
N_LINES = 8192
N_CORES = 8
TOK = N_LINES // N_CORES    # 1024 tokens per core
ROWS = TOK + HALO           # 1088 combined rows per core
GRP = 512                   # tokens per attention group
NG = TOK // GRP             # 2 groups
# key chunks (start, end), valid token windows (w0, w1), mask index
CHUNKS = [(0, 128, 0, 128, 0), (128, 256, 64, 256, 1), (256, 384, 192, 384, 1),
          (384, 512, 320, 512, 1), (512, 576, 448, 512, 2)]
VBLK = HEAD_DIM + 1         # 97: v head block + ones column
KC = EMBED_DIM // 128       # 6 contraction chunks of 128
DC = KC // 2                # 3 DoubleRow chunks of 256
NVC = (ROWS + 127) // 128   # 9 v row-chunks (8x128 + 64)
WS = 64.0                   # 2^6 fp8 weight pre-scale


def _build_program():
    import concourse.bacc as bacc
    import concourse.mybir as mybir
    from concourse.tile import TileContext

    f32 = mybir.dt.float32
    bf16 = mybir.dt.bfloat16
    f8 = mybir.dt.float8e4
    DR = mybir.MatmulPerfMode.DoubleRow
    ACT = mybir.ActivationFunctionType
    ALU = mybir.AluOpType
    D = EMBED_DIM

    nc = bacc.Bacc("TRN2", target_bir_lowering=False, debug=False,
                   enable_asserts=False, num_devices=N_CORES)

    x8 = nc.dram_tensor("x8", [D, ROWS], f8, kind="ExternalInput")
    wq8 = nc.dram_tensor("wq8", [D, D], f8, kind="ExternalInput")
    wk8 = nc.dram_tensor("wk8", [D, D], f8, kind="ExternalInput")
    wv8 = nc.dram_tensor("wv8", [D, D], f8, kind="ExternalInput")
    wo8 = nc.dram_tensor("wo8", [D, D], f8, kind="ExternalInput")
    bq = nc.dram_tensor("bq", [HEAD_DIM, NUM_HEADS], f32, kind="ExternalInput")
    bo2 = nc.dram_tensor("bo2", [128, KC], f32, kind="ExternalInput")
    m0 = nc.dram_tensor("m0", [128, 128], bf16, kind="ExternalInput")
    m1 = nc.dram_tensor("m1", [128, 192], bf16, kind="ExternalInput")
    m2 = nc.dram_tensor("m2", [64, 64], bf16, kind="ExternalInput")
    out = nc.dram_tensor("out", [D, TOK], bf16, kind="ExternalOutput")

    def pair(tile, dc, lo, sz, inner):
        # [128, 2, sz] DoubleRow view of contraction chunk pair dc
        return tile[:, 2 * dc * inner:(2 * dc + 2) * inner] \
            .rearrange("p (i n) -> p i n", i=2)[:, :, lo:lo + sz]

    with TileContext(nc) as tc:
        with tc.tile_pool(name="persist", bufs=1) as pers:
            qTh = [pers.tile([HEAD_DIM, TOK], bf16, name=f"qTh{h}")
                   for h in range(NUM_HEADS)]
            kTh = [pers.tile([HEAD_DIM, ROWS], bf16, name=f"kTh{h}")
                   for h in range(NUM_HEADS)]
            vt = [pers.tile([128, NUM_HEADS * VBLK], bf16, name=f"vt{r}")
                  for r in range(NVC)]
            ctxH = [pers.tile([HEAD_DIM, NUM_HEADS * GRP], f8, name=f"ctxH{g}")
                    for g in range(NG)]
            bqt = pers.tile([HEAD_DIM, NUM_HEADS], f32)
            bo2t = pers.tile([128, KC], f32)
            mk0 = pers.tile([128, 128], bf16)
            mk1 = pers.tile([128, 192], bf16)
            mk2 = pers.tile([64, 64], bf16)
            masks = [mk0, mk1, mk2]
            xt = pers.tile([128, KC * ROWS], f8, name="xt")
            wvt = pers.tile([128, KC * D], f8, name="w_v")
            wqt = pers.tile([128, KC * D], f8, name="w_q")
            wkt = pers.tile([128, KC * D], f8, name="w_k")
            wot = pers.tile([HEAD_DIM, NUM_HEADS * D], f8, name="w_o")

            for r in range(NVC):
                rows = min(128, ROWS - 128 * r)
                dst = vt[r][0:rows, :].rearrange("p (b c) -> p b c", c=VBLK)
                nc.gpsimd.memset(dst[:, :, HEAD_DIM:VBLK], 1.0)

            # 3-way DMA queue split: sync + scalar (HWDGE) + gpsimd (SWDGE)
            def dma_w(eng, tile, src, c):
                eng.dma_start(tile[:, c * D:(c + 1) * D],
                              src.ap()[c * 128:(c + 1) * 128, :])

            def dma_x(eng, c):
                eng.dma_start(xt[:, c * ROWS:(c + 1) * ROWS],
                              x8.ap()[c * 128:(c + 1) * 128, :])

            # sync: v inputs first, then x evens, wk evens, wo evens
            dma_w(nc.sync, wvt, wv8, 0)
            dma_w(nc.sync, wvt, wv8, 1)
            dma_x(nc.sync, 0)
            dma_w(nc.sync, wvt, wv8, 2)
            dma_x(nc.sync, 2)
            dma_w(nc.sync, wvt, wv8, 3)
            dma_x(nc.sync, 4)
            for c in (0, 2, 4):
                dma_w(nc.sync, wkt, wk8, c)
            for h in (0, 2, 4, 6):
                nc.sync.dma_start(wot[:, h * D:(h + 1) * D],
                                  wo8.ap()[h * HEAD_DIM:(h + 1) * HEAD_DIM, :])
            nc.sync.dma_start(bqt[:], bq.ap())
            nc.sync.dma_start(mk0[:], m0.ap())
            nc.sync.dma_start(mk1[:], m1.ap())
            nc.sync.dma_start(mk2[:], m2.ap())
            nc.sync.dma_start(bo2t[:], bo2.ap())
            # scalar: q weights + wk odds (ACT queue is idle early)
            for c in range(KC):
                dma_w(nc.scalar, wqt, wq8, c)
            for c in (1, 3, 5):
                dma_w(nc.scalar, wkt, wk8, c)
            # gpsimd: x odds + remaining v/o weights
            dma_x(nc.gpsimd, 1)
            dma_x(nc.gpsimd, 3)
            dma_x(nc.gpsimd, 5)
            dma_w(nc.gpsimd, wvt, wv8, 4)
            dma_w(nc.gpsimd, wvt, wv8, 5)
            for h in (1, 3, 5, 7):
                nc.gpsimd.dma_start(wot[:, h * D:(h + 1) * D],
                                    wo8.ap()[h * HEAD_DIM:(h + 1) * HEAD_DIM, :])

            with tc.tile_pool(name="vpsum", bufs=2, space="PSUM") as vpsum, \
                 tc.tile_pool(name="ppsum", bufs=2, space="PSUM") as ppsum:
                # ---- v projection (x-stationary, fp8 DoubleRow)
                for r in range(NVC):
                    rows = min(128, ROWS - 128 * r)
                    pv0 = vpsum.tile([128, 512], f32, tag="pv0", name="pv0")
                    pv1 = vpsum.tile([128, 256], f32, tag="pv1", name="pv1")
                    for dc in range(DC):
                        lhs = pair(xt, dc, 128 * r, rows, ROWS)
                        nc.tensor.matmul(pv0[0:rows, :], lhs,
                                         pair(wvt, dc, 0, 512, D),
                                         start=(dc == 0), stop=(dc == DC - 1),
                                         perf_mode=DR)
                        nc.tensor.matmul(pv1[0:rows, :], lhs,
                                         pair(wvt, dc, 512, 256, D),
                                         start=(dc == 0), stop=(dc == DC - 1),
                                         perf_mode=DR)
                    dst = vt[r][0:rows, :].rearrange("p (b c) -> p b c", c=VBLK)
                    nc.scalar.activation(
                        dst[:, 0:5, 0:HEAD_DIM],
                        pv0[0:rows, 0:5 * HEAD_DIM]
                        .rearrange("p (b c) -> p b c", c=HEAD_DIM),
                        ACT.Copy, scale=1.0 / WS)
                    # head 5 straddles the 512 boundary: cols 480:512 | 0:64
                    nc.vector.tensor_scalar_mul(dst[:, 5, 0:32],
                                                pv0[0:rows, 480:512], 1.0 / WS)
                    nc.vector.tensor_scalar_mul(dst[:, 5, 32:HEAD_DIM],
                                                pv1[0:rows, 0:64], 1.0 / WS)
                    nc.vector.tensor_scalar_mul(
                        dst[:, 6:8, 0:HEAD_DIM],
                        pv1[0:rows, 64:64 + 2 * HEAD_DIM]
                        .rearrange("p (b c) -> p b c", c=HEAD_DIM), 1.0 / WS)

                # ---- q / k projections (weight-stationary, fp8 DoubleRow)
                for h in range(NUM_HEADS):
                    for n0 in (0, 512):
                        ps = ppsum.tile([HEAD_DIM, 512], f32, tag="pqk",
                                        name="ps_q")
                        for dc in range(DC):
                            nc.tensor.matmul(
                                ps[:],
                                pair(wqt, dc, h * HEAD_DIM, HEAD_DIM, D),
                                pair(xt, dc, OVERLAP + n0, 512, ROWS),
                                start=(dc == 0), stop=(dc == DC - 1),
                                perf_mode=DR)
                        # qTh keeps the 2^6 weight scale; exp absorbs it
                        nc.scalar.activation(qTh[h][:, n0:n0 + 512], ps[:],
                                             ACT.Identity,
                                             bias=bqt[:, h:h + 1], scale=1.0)
                    for n0, sz in ((0, 512), (512, 512), (1024, 64)):
                        ps = ppsum.tile([HEAD_DIM, 512], f32, tag="pqk",
                                        name="ps_k")
                        for dc in range(DC):
                            nc.tensor.matmul(
                                ps[:, 0:sz],
                                pair(wkt, dc, h * HEAD_DIM, HEAD_DIM, D),
                                pair(xt, dc, n0, sz, ROWS),
                                start=(dc == 0), stop=(dc == DC - 1),
                                perf_mode=DR)
                        nc.vector.tensor_copy(kTh[h][:, n0:n0 + sz],
                                              ps[:, 0:sz])

            # ---- attention + normalization + out-projection, pipelined
            with tc.tile_pool(name="apool", bufs=2) as apool, \
                 tc.tile_pool(name="opool", bufs=2) as opool, \
                 tc.tile_pool(name="apsum", bufs=2, space="PSUM") as apsum, \
                 tc.tile_pool(name="opsum", bufs=2, space="PSUM") as opsum:

                def attention_head(g, h, den):
                    ctx_ps = apsum.tile([VBLK, GRP], f32, tag="ctx",
                                        name="ctx_ps", bufs=3)
                    for c, (k0, k1, w0, w1, mi) in enumerate(CHUNKS):
                        ksz = k1 - k0
                        win = w1 - w0
                        s_ps = apsum.tile([128, 192], f32, tag="s", name="s_ps",
                                          bufs=3)
                        nc.tensor.matmul(
                            s_ps[0:ksz, 0:win],
                            kTh[h][:, GRP * g + k0: GRP * g + k1],
                            qTh[h][:, GRP * g + w0: GRP * g + w1],
                            start=True, stop=True)
                        ex = apool.tile([128, 192], bf16, tag="ex", name="ex",
                                        bufs=8)
                        # scores carry 2^12 from the two 2^6 weight scales
                        nc.scalar.activation(
                            ex[0:ksz, 0:win], s_ps[0:ksz, 0:win],
                            ACT.Exp, scale=1.0 / (WS * WS))
                        nc.vector.tensor_tensor(
                            out=ex[0:ksz, 0:win], in0=ex[0:ksz, 0:win],
                            in1=masks[mi][0:ksz, :], op=ALU.mult)
                        nc.tensor.matmul(
                            ctx_ps[:, w0:w1],
                            vt[4 * g + c][0:ksz, h * VBLK:(h + 1) * VBLK],
                            ex[0:ksz, 0:win],
                            start=(c == 0), stop=(c == len(CHUNKS) - 1),
                            skip_group_check=True)
                    # stage the denominator row at partition 0
                    nc.scalar.copy(den[:, (h % 2) * GRP:(h % 2 + 1) * GRP],
                                   ctx_ps[HEAD_DIM:VBLK, :])
                    return ctx_ps

                def normalize_pair(g, hp, den, ctxs):
                    # 1/denom on DVE (avoids exp<->ln ACT table thrash)
                    rre = apool.tile([1, 2 * GRP], f32, tag="rre", name="rre",
                                     bufs=2)
                    rrb = apool.tile([1, 2 * GRP], bf16, tag="rrb", name="rrb",
                                     bufs=2)
                    nc.vector.reciprocal_approx_fast(rre[:], den[:])
                    nc.vector.tensor_copy(rrb[:], rre[:])
                    for j in range(2):
                        h = 2 * hp + j
                        rdb = apool.tile([HEAD_DIM, GRP], bf16, tag="rdb",
                                         name="rdb", bufs=4)
                        nc.gpsimd.partition_broadcast(
                            rdb[:], rrb[:, j * GRP:(j + 1) * GRP])
                        # fused normalize + fp8 cast drain
                        nc.vector.scalar_tensor_tensor(
                            out=ctxH[g][:, h * GRP:(h + 1) * GRP],
                            in0=ctxs[j][0:HEAD_DIM, :], scalar=0.0, in1=rdb[:],
                            op0=ALU.bypass, op1=ALU.mult)

                def outproj_half(g):
                    for dc in range(KC):
                        op = opsum.tile([128, 512], f32, tag="po", name="ps_o",
                                        bufs=2)
                        for hp in range(NUM_HEADS // 2):
                            nc.tensor.matmul(
                                op[:],
                                pair(wot, hp, dc * 128, 128, D),
                                ctxH[g][:, 2 * hp * GRP:(2 * hp + 2) * GRP]
                                .rearrange("p (i n) -> p i n", i=2),
                                start=(hp == 0), stop=(hp == NUM_HEADS // 2 - 1),
                                perf_mode=DR)
                        ost = opool.tile([128, 512], bf16, tag="ost", name="ost")
                        nc.scalar.activation(
                            ost[:], op[:], ACT.Identity,
                            bias=bo2t[:, dc:dc + 1], scale=1.0 / WS)
                        # split each chunk across two queues to halve the
                        # per-chunk transfer time (each queue is ~25GB/s)
                        qs = (nc.sync, nc.gpsimd, nc.scalar)
                        for i in range(2):
                            qs[(2 * (g * KC + dc) + i) % 3].dma_start(
                                out.ap()[dc * 128 + 64 * i:
                                         dc * 128 + 64 * (i + 1),
                                         g * 512:(g + 1) * 512],
                                ost[64 * i:64 * (i + 1), :])

                for hp in range(NUM_HEADS // 2):
                    for g in range(NG):
                        den = apool.tile([1, 2 * GRP], f32, tag="den",
                                         name="den", bufs=2)
                        ctxs = [attention_head(g, 2 * hp + j, den)
                                for j in range(2)]
                        normalize_pair(g, hp, den, ctxs)
                for g in range(NG):
                    outproj_half(g)
    nc.compile()
    return nc


_program_cache = {}


def _get_program():
    if "nc" not in _program_cache:
        _program_cache["nc"] = _build_program()
    return _program_cache["nc"]


def _host_masks():
    # Three mask patterns: d = key - token offset within the chunk window.
    # m0 (first chunk): d = kk - mm; m1/m2 (later chunks): d = kk - mm + 64.
    import ml_dtypes
    masks = []
    for (nk, nw, off) in ((128, 128, 0), (128, 192, HALO), (64, 64, HALO)):
        kk, mm = np.meshgrid(np.arange(nk), np.arange(nw), indexing="ij")
        d = kk - mm + off
        valid = (d >= 0) & (d <= HALO) & (d % 4 == 0) & (d != OVERLAP)
        masks.append(valid.astype(ml_dtypes.bfloat16))
    return masks


def kernel(main, begin, end, in_proj_w, in_proj_b, out_proj_w, out_proj_b):
    import ml_dtypes
    from concourse.bass_utils import run_bass_kernel_spmd

    f8np = ml_dtypes.float8_e4m3

    main = np.asarray(main, np.float32)
    begin = np.asarray(begin, np.float32)
    end = np.asarray(end, np.float32)
    in_proj_w = np.asarray(in_proj_w, np.float32)
    in_proj_b = np.asarray(in_proj_b, np.float32)
    out_proj_w = np.asarray(out_proj_w, np.float32)
    out_proj_b = np.asarray(out_proj_b, np.float32)

    D = EMBED_DIM
    scale = HEAD_DIM ** -0.5
    wq, wk, wv = in_proj_w[:D], in_proj_w[D:2 * D], in_proj_w[2 * D:]
    bq_, bv = in_proj_b[:D], in_proj_b[2 * D:3 * D]
    combined = np.concatenate([begin, main, end], axis=0)  # [N + 64, D]

    wq8 = np.ascontiguousarray(wq.T * (scale * WS)).astype(f8np)
    wk8 = np.ascontiguousarray(wk.T * WS).astype(f8np)
    wv8 = np.ascontiguousarray(wv.T * WS).astype(f8np)
    wo8 = np.ascontiguousarray(out_proj_w.T * WS).astype(f8np)
    bq_heads = np.ascontiguousarray(
        (bq_ * (scale * WS)).reshape(NUM_HEADS, HEAD_DIM).T)
    bo2 = out_proj_w @ bv + out_proj_b                      # [768]
    bo2_chunks = np.ascontiguousarray(bo2.reshape(KC, 128).T)
    masks = _host_masks()

    shared = {
        "wq8": wq8, "wk8": wk8, "wv8": wv8, "wo8": wo8,
        "bq": bq_heads, "bo2": bo2_chunks,
        "m0": masks[0], "m1": masks[1], "m2": masks[2],
    }
    in_maps = []
    for c in range(N_CORES):
        x8c = np.ascontiguousarray(
            combined[c * TOK: c * TOK + ROWS].T).astype(f8np)
        in_maps.append({**shared, "x8": x8c})

    nc = _get_program()
    res = run_bass_kernel_spmd(nc, in_maps, core_ids=list(range(N_CORES)),
                               **_program_cache.get("run_kwargs", {}))
    _program_cache["last_result"] = res

    outp = np.empty((N_LINES, 2 * D), np.float32)
    outp[:, :D] = main
    for c in range(N_CORES):
        outp[c * TOK:(c + 1) * TOK, D:] = \
            res.results[c]["out"].astype(np.float32).T
    return outp
